# revision 6
# baseline (speedup 1.0000x reference)
"""SwinBlock kernel: data-parallel across 8 NeuronCores.

Shards batch B=64 across 8 devices (8 images/core), replicates the ~0.2MB
parameter set, runs the full Swin block per shard via PJRT on trn2, and
gathers the full [64, 3136, 96] output. Shapes hardcoded per contract.
"""
import functools

import numpy as np
import jax
import jax.numpy as jnp

B, H, W, C = 64, 3136 // 56, 56, 96  # H derived = 56
H = 56
WS, SS, NH = 7, 3, 3
L2 = WS * WS
NW = (H // WS) * (W // WS)
N_CORES = 8


def _attn_mask_np() -> np.ndarray:
    img = np.zeros((H, W), np.float32)
    cnt = 0
    sl = (slice(0, -WS), slice(-WS, -SS), slice(-SS, None))
    for hs in sl:
        for ws in sl:
            img[hs, ws] = cnt
            cnt += 1
    mw = img.reshape(H // WS, WS, W // WS, WS).transpose(0, 2, 1, 3).reshape(NW, L2)
    am = mw[:, None, :] - mw[:, :, None]
    return np.where(am != 0, -100.0, 0.0).astype(np.float32)  # [NW, L2, L2]


_MASK = _attn_mask_np()


def _ln(x, g, b):
    m = x.mean(-1, keepdims=True)
    v = ((x - m) ** 2).mean(-1, keepdims=True)
    return (x - m) / jnp.sqrt(v + 1e-5) * g + b


def _mm(a, b):
    """bf16 matmul with fp32 accumulation: 4x PE throughput vs fp32 on trn2."""
    return jnp.matmul(a.astype(jnp.bfloat16), b.astype(jnp.bfloat16),
                      preferred_element_type=jnp.float32)


def _ein(spec, a, b):
    return jnp.einsum(spec, a.astype(jnp.bfloat16), b.astype(jnp.bfloat16),
                      preferred_element_type=jnp.float32)


def _swin_block_shard(x, qkv_w, qkv_b, attn_bias, proj_w, proj_b, ln1_g, ln1_b,
                      ln2_g, ln2_b, mlp_w1, mlp_b1, mlp_w2, mlp_b2, mask):
    """x: [Bb, H*W, C] one shard; returns same shape."""
    Bb = x.shape[0]
    Dh = C // NH
    shortcut = x
    h = _ln(x, ln1_g, ln1_b).reshape(Bb, H, W, C)
    h = jnp.roll(h, (-SS, -SS), (1, 2))
    h = h.reshape(Bb, H // WS, WS, W // WS, WS, C).transpose(0, 1, 3, 2, 4, 5)
    h = h.reshape(Bb * NW, L2, C)
    qkv = (_mm(h, qkv_w) + qkv_b).reshape(Bb * NW, L2, 3, NH, Dh).transpose(2, 0, 3, 1, 4)
    q, k, v = qkv[0], qkv[1], qkv[2]
    scale = Dh ** -0.5
    attn = _ein('bhqd,bhkd->bhqk', q * scale, k) + attn_bias
    attn = attn.reshape(Bb, NW, NH, L2, L2) + mask[None, :, None]
    attn = jax.nn.softmax(attn.reshape(Bb * NW, NH, L2, L2), axis=-1)
    o = _ein('bhqk,bhkd->bqhd', attn, v).reshape(Bb * NW, L2, C)
    o = _mm(o, proj_w) + proj_b
    o = o.reshape(Bb, H // WS, W // WS, WS, WS, C).transpose(0, 1, 3, 2, 4, 5)
    o = o.reshape(Bb, H, W, C)
    o = jnp.roll(o, (SS, SS), (1, 2)).reshape(Bb, H * W, C)
    x = shortcut + o
    y = _ln(x, ln2_g, ln2_b)
    y = _mm(jax.nn.gelu(_mm(y, mlp_w1) + mlp_b1, approximate=False), mlp_w2) + mlp_b2
    return x + y


@functools.lru_cache(maxsize=1)
def _n_shards() -> int:
    return min(N_CORES, len(jax.devices()))


@functools.lru_cache(maxsize=1)
def _get_pmapped():
    return jax.pmap(
        _swin_block_shard,
        in_axes=(0,) + (None,) * 14,
        devices=jax.devices()[:_n_shards()],
    )


_PARAM_ORDER = ("qkv_w", "qkv_b", "attn_bias", "proj_w", "proj_b", "ln1_g",
                "ln1_b", "ln2_g", "ln2_b", "mlp_w1", "mlp_b1", "mlp_w2",
                "mlp_b2")


def stage(x, **params):
    """Pre-stage shards + params on device (for device-only timing)."""
    x = np.asarray(x, dtype=np.float32)
    nsh = _n_shards()
    shards = jnp.asarray(
        x.reshape(nsh, x.shape[0] // nsh, x.shape[1], x.shape[2]))
    ps = tuple(jnp.asarray(params[k], jnp.float32) for k in _PARAM_ORDER)
    return (shards,) + ps + (jnp.asarray(_MASK),)


def run_staged(staged):
    return _get_pmapped()(*staged)


def kernel(x, qkv_w, qkv_b, attn_bias, proj_w, proj_b, ln1_g, ln1_b,
           ln2_g, ln2_b, mlp_w1, mlp_b1, mlp_w2, mlp_b2):
    x = np.asarray(x, dtype=np.float32)
    Bfull = x.shape[0]
    nsh = _n_shards()
    shards = x.reshape(nsh, Bfull // nsh, x.shape[1], x.shape[2])
    fn = _get_pmapped()
    out = fn(
        jnp.asarray(shards),
        jnp.asarray(qkv_w, jnp.float32), jnp.asarray(qkv_b, jnp.float32),
        jnp.asarray(attn_bias, jnp.float32),
        jnp.asarray(proj_w, jnp.float32), jnp.asarray(proj_b, jnp.float32),
        jnp.asarray(ln1_g, jnp.float32), jnp.asarray(ln1_b, jnp.float32),
        jnp.asarray(ln2_g, jnp.float32), jnp.asarray(ln2_b, jnp.float32),
        jnp.asarray(mlp_w1, jnp.float32), jnp.asarray(mlp_b1, jnp.float32),
        jnp.asarray(mlp_w2, jnp.float32), jnp.asarray(mlp_b2, jnp.float32),
        jnp.asarray(_MASK),
    )
    out = np.asarray(out, dtype=np.float32)
    return out.reshape(Bfull, x.shape[1], x.shape[2])


# revision 7
# speedup vs baseline: 1.0149x; 1.0149x over previous
"""SwinBlock kernel: data-parallel across 8 NeuronCores.

Shards batch B=64 across 8 devices (8 images/core), replicates the ~0.2MB
parameter set, runs the full Swin block per shard via PJRT on trn2, and
gathers the full [64, 3136, 96] output. Shapes hardcoded per contract.
"""
import functools

import numpy as np
import jax
import jax.numpy as jnp

B, H, W, C = 64, 3136 // 56, 56, 96  # H derived = 56
H = 56
WS, SS, NH = 7, 3, 3
L2 = WS * WS
NW = (H // WS) * (W // WS)
N_CORES = 8


def _attn_mask_np() -> np.ndarray:
    img = np.zeros((H, W), np.float32)
    cnt = 0
    sl = (slice(0, -WS), slice(-WS, -SS), slice(-SS, None))
    for hs in sl:
        for ws in sl:
            img[hs, ws] = cnt
            cnt += 1
    mw = img.reshape(H // WS, WS, W // WS, WS).transpose(0, 2, 1, 3).reshape(NW, L2)
    am = mw[:, None, :] - mw[:, :, None]
    return np.where(am != 0, -100.0, 0.0).astype(np.float32)  # [NW, L2, L2]


_MASK = _attn_mask_np()

def _perm_np() -> np.ndarray:
    idx = np.arange(H * W, dtype=np.int32).reshape(H, W)
    idx = np.roll(idx, (-SS, -SS), (0, 1))
    idx = idx.reshape(H // WS, WS, W // WS, WS).transpose(0, 2, 1, 3)
    return idx.reshape(-1)


_PERM = _perm_np()
_INV_PERM = np.argsort(_PERM).astype(np.int32)


def _ln(x, g, b):
    m = x.mean(-1, keepdims=True)
    v = ((x - m) ** 2).mean(-1, keepdims=True)
    return (x - m) / jnp.sqrt(v + 1e-5) * g + b


def _mm(a, b):
    """bf16 matmul with fp32 accumulation: 4x PE throughput vs fp32 on trn2."""
    return jnp.matmul(a.astype(jnp.bfloat16), b.astype(jnp.bfloat16),
                      preferred_element_type=jnp.float32)


def _ein(spec, a, b):
    return jnp.einsum(spec, a.astype(jnp.bfloat16), b.astype(jnp.bfloat16),
                      preferred_element_type=jnp.float32)


def _swin_block_shard(x, qkv_w, qkv_b, attn_bias, proj_w, proj_b, ln1_g, ln1_b,
                      ln2_g, ln2_b, mlp_w1, mlp_b1, mlp_w2, mlp_b2, mask, perm, inv_perm):
    """x: [Bb, H*W, C] one shard; returns same shape."""
    Bb = x.shape[0]
    Dh = C // NH
    shortcut = x
    h = _ln(x, ln1_g, ln1_b)
    h = jnp.take(h, perm, axis=1).reshape(Bb * NW, L2, C)
    qkv = (_mm(h, qkv_w) + qkv_b).reshape(Bb * NW, L2, 3, NH, Dh).transpose(2, 0, 3, 1, 4)
    q, k, v = qkv[0], qkv[1], qkv[2]
    scale = Dh ** -0.5
    attn = _ein('bhqd,bhkd->bhqk', q * scale, k) + attn_bias
    attn = attn.reshape(Bb, NW, NH, L2, L2) + mask[None, :, None]
    attn = jax.nn.softmax(attn.reshape(Bb * NW, NH, L2, L2), axis=-1)
    o = _ein('bhqk,bhkd->bqhd', attn, v).reshape(Bb * NW, L2, C)
    o = _mm(o, proj_w) + proj_b
    o = jnp.take(o.reshape(Bb, H * W, C), inv_perm, axis=1)
    x = shortcut + o
    y = _ln(x, ln2_g, ln2_b)
    y = _mm(jax.nn.gelu(_mm(y, mlp_w1) + mlp_b1, approximate=False), mlp_w2) + mlp_b2
    return x + y


@functools.lru_cache(maxsize=1)
def _n_shards() -> int:
    return min(N_CORES, len(jax.devices()))


@functools.lru_cache(maxsize=1)
def _get_pmapped():
    return jax.pmap(
        _swin_block_shard,
        in_axes=(0,) + (None,) * 16,
        devices=jax.devices()[:_n_shards()],
    )


_PARAM_ORDER = ("qkv_w", "qkv_b", "attn_bias", "proj_w", "proj_b", "ln1_g",
                "ln1_b", "ln2_g", "ln2_b", "mlp_w1", "mlp_b1", "mlp_w2",
                "mlp_b2")


def stage(x, **params):
    """Pre-stage shards + params on device (for device-only timing)."""
    x = np.asarray(x, dtype=np.float32)
    nsh = _n_shards()
    shards = jnp.asarray(
        x.reshape(nsh, x.shape[0] // nsh, x.shape[1], x.shape[2]))
    ps = tuple(jnp.asarray(params[k], jnp.float32) for k in _PARAM_ORDER)
    return (shards,) + ps + (jnp.asarray(_MASK), jnp.asarray(_PERM), jnp.asarray(_INV_PERM))


def run_staged(staged):
    return _get_pmapped()(*staged)


def kernel(x, qkv_w, qkv_b, attn_bias, proj_w, proj_b, ln1_g, ln1_b,
           ln2_g, ln2_b, mlp_w1, mlp_b1, mlp_w2, mlp_b2):
    x = np.asarray(x, dtype=np.float32)
    Bfull = x.shape[0]
    nsh = _n_shards()
    shards = x.reshape(nsh, Bfull // nsh, x.shape[1], x.shape[2])
    fn = _get_pmapped()
    out = fn(
        jnp.asarray(shards),
        jnp.asarray(qkv_w, jnp.float32), jnp.asarray(qkv_b, jnp.float32),
        jnp.asarray(attn_bias, jnp.float32),
        jnp.asarray(proj_w, jnp.float32), jnp.asarray(proj_b, jnp.float32),
        jnp.asarray(ln1_g, jnp.float32), jnp.asarray(ln1_b, jnp.float32),
        jnp.asarray(ln2_g, jnp.float32), jnp.asarray(ln2_b, jnp.float32),
        jnp.asarray(mlp_w1, jnp.float32), jnp.asarray(mlp_b1, jnp.float32),
        jnp.asarray(mlp_w2, jnp.float32), jnp.asarray(mlp_b2, jnp.float32),
        jnp.asarray(_MASK), jnp.asarray(_PERM), jnp.asarray(_INV_PERM),
    )
    out = np.asarray(out, dtype=np.float32)
    return out.reshape(Bfull, x.shape[1], x.shape[2])


# revision 8
# speedup vs baseline: 1.0571x; 1.0416x over previous
"""SwinBlock kernel: data-parallel across 8 NeuronCores.

Shards batch B=64 across 8 devices (8 images/core), replicates the ~0.2MB
parameter set, runs the full Swin block per shard via PJRT on trn2, and
gathers the full [64, 3136, 96] output. Shapes hardcoded per contract.
"""
import functools

import numpy as np
import jax
import jax.numpy as jnp

B, H, W, C = 64, 3136 // 56, 56, 96  # H derived = 56
H = 56
WS, SS, NH = 7, 3, 3
L2 = WS * WS
NW = (H // WS) * (W // WS)
N_CORES = 8


def _attn_mask_np() -> np.ndarray:
    img = np.zeros((H, W), np.float32)
    cnt = 0
    sl = (slice(0, -WS), slice(-WS, -SS), slice(-SS, None))
    for hs in sl:
        for ws in sl:
            img[hs, ws] = cnt
            cnt += 1
    mw = img.reshape(H // WS, WS, W // WS, WS).transpose(0, 2, 1, 3).reshape(NW, L2)
    am = mw[:, None, :] - mw[:, :, None]
    return np.where(am != 0, -100.0, 0.0).astype(np.float32)  # [NW, L2, L2]


_MASK = _attn_mask_np()

def _perm_np() -> np.ndarray:
    idx = np.arange(H * W, dtype=np.int32).reshape(H, W)
    idx = np.roll(idx, (-SS, -SS), (0, 1))
    idx = idx.reshape(H // WS, WS, W // WS, WS).transpose(0, 2, 1, 3)
    return idx.reshape(-1)


_PERM = _perm_np()
_INV_PERM = np.argsort(_PERM).astype(np.int32)



def _fold_params(qkv_w, qkv_b, attn_bias):
    """Fold q-scale into qkv weights; combine bias+mask into one add."""
    qkv_w = np.asarray(qkv_w, np.float32).copy()
    qkv_b = np.asarray(qkv_b, np.float32).copy()
    scale = (C // NH) ** -0.5
    qkv_w[:, :C] *= scale
    qkv_b[:C] *= scale
    biasmask = (np.asarray(attn_bias, np.float32)[None] +
                _MASK[:, None])  # [NW, NH, L2, L2]
    return qkv_w, qkv_b, biasmask


def _ln(x, g, b):
    m = x.mean(-1, keepdims=True)
    v = ((x - m) ** 2).mean(-1, keepdims=True)
    return (x - m) / jnp.sqrt(v + 1e-5) * g + b


def _mm(a, b):
    """bf16 matmul with fp32 accumulation: 4x PE throughput vs fp32 on trn2."""
    return jnp.matmul(a.astype(jnp.bfloat16), b.astype(jnp.bfloat16),
                      preferred_element_type=jnp.float32)


def _ein(spec, a, b):
    return jnp.einsum(spec, a.astype(jnp.bfloat16), b.astype(jnp.bfloat16),
                      preferred_element_type=jnp.float32)


def _swin_block_shard(x, qkv_w, qkv_b, attn_bias, proj_w, proj_b, ln1_g, ln1_b,
                      ln2_g, ln2_b, mlp_w1, mlp_b1, mlp_w2, mlp_b2, mask, perm, inv_perm):
    """x: [Bb, H*W, C] one shard; returns same shape."""
    Bb = x.shape[0]
    Dh = C // NH
    shortcut = x
    h = _ln(x, ln1_g, ln1_b)
    h = jnp.take(h, perm, axis=1).reshape(Bb * NW, L2, C)
    qkv = (_mm(h, qkv_w) + qkv_b).reshape(Bb * NW, L2, 3, NH, Dh).transpose(2, 0, 3, 1, 4)
    q, k, v = qkv[0], qkv[1], qkv[2]
    attn = _ein('bhqd,bhkd->bhqk', q, k)
    attn = attn.reshape(Bb, NW, NH, L2, L2) + attn_bias[None]
    attn = jax.nn.softmax(attn.reshape(Bb * NW, NH, L2, L2), axis=-1)
    o = _ein('bhqk,bhkd->bqhd', attn, v).reshape(Bb * NW, L2, C)
    o = _mm(o, proj_w) + proj_b
    o = jnp.take(o.reshape(Bb, H * W, C), inv_perm, axis=1)
    x = shortcut + o
    y = _ln(x, ln2_g, ln2_b)
    y = _mm(jax.nn.gelu(_mm(y, mlp_w1) + mlp_b1, approximate=False), mlp_w2) + mlp_b2
    return x + y


@functools.lru_cache(maxsize=1)
def _n_shards() -> int:
    return min(N_CORES, len(jax.devices()))


@functools.lru_cache(maxsize=1)
def _get_pmapped():
    return jax.pmap(
        _swin_block_shard,
        in_axes=(0,) + (None,) * 16,
        devices=jax.devices()[:_n_shards()],
    )


_PARAM_ORDER = ("qkv_w", "qkv_b", "attn_bias", "proj_w", "proj_b", "ln1_g",
                "ln1_b", "ln2_g", "ln2_b", "mlp_w1", "mlp_b1", "mlp_w2",
                "mlp_b2")


def stage(x, **params):
    """Pre-stage shards + params on device (for device-only timing)."""
    x = np.asarray(x, dtype=np.float32)
    nsh = _n_shards()
    shards = jnp.asarray(
        x.reshape(nsh, x.shape[0] // nsh, x.shape[1], x.shape[2]))
    params = dict(params)
    params["qkv_w"], params["qkv_b"], params["attn_bias"] = _fold_params(
        params["qkv_w"], params["qkv_b"], params["attn_bias"])
    ps = tuple(jnp.asarray(params[k], jnp.float32) for k in _PARAM_ORDER)
    return (shards,) + ps + (jnp.asarray(_MASK), jnp.asarray(_PERM), jnp.asarray(_INV_PERM))


def run_staged(staged):
    return _get_pmapped()(*staged)


def kernel(x, qkv_w, qkv_b, attn_bias, proj_w, proj_b, ln1_g, ln1_b,
           ln2_g, ln2_b, mlp_w1, mlp_b1, mlp_w2, mlp_b2):
    x = np.asarray(x, dtype=np.float32)
    Bfull = x.shape[0]
    nsh = _n_shards()
    shards = x.reshape(nsh, Bfull // nsh, x.shape[1], x.shape[2])
    fn = _get_pmapped()
    qkv_w, qkv_b, attn_bias = _fold_params(qkv_w, qkv_b, attn_bias)
    out = fn(
        jnp.asarray(shards),
        jnp.asarray(qkv_w, jnp.float32), jnp.asarray(qkv_b, jnp.float32),
        jnp.asarray(attn_bias, jnp.float32),
        jnp.asarray(proj_w, jnp.float32), jnp.asarray(proj_b, jnp.float32),
        jnp.asarray(ln1_g, jnp.float32), jnp.asarray(ln1_b, jnp.float32),
        jnp.asarray(ln2_g, jnp.float32), jnp.asarray(ln2_b, jnp.float32),
        jnp.asarray(mlp_w1, jnp.float32), jnp.asarray(mlp_b1, jnp.float32),
        jnp.asarray(mlp_w2, jnp.float32), jnp.asarray(mlp_b2, jnp.float32),
        jnp.asarray(_MASK), jnp.asarray(_PERM), jnp.asarray(_INV_PERM),
    )
    out = np.asarray(out, dtype=np.float32)
    return out.reshape(Bfull, x.shape[1], x.shape[2])
